# revision 46
# baseline (speedup 1.0000x reference)
"""Trainium2 Bass kernel for CompressiveMemory (Infini-attention style).

Sharding: 8 cores = 4 batch x 2 head-groups (8 heads each). The reference's
`att.reshape(B, SEG, H*dv)` is a raw view of a (B,H,SEG,dv) tensor, so each
block of 16 output rows depends on exactly one head: head-sharding needs no
cross-core reduction, only row scattering (done on host).

Per-core kernel. Layouts put matmul contractions on the partition dim; heads
are processed in pairs (even head on partitions 0:64, odd on 64:128) so the
K=64 matmuls (scores, att_mem) auto-place into PE row groups 0/64 and
co-execute, and the M=64 state updates co-execute via column groups.

PSUM (8 banks):
  - "big" ring, 3 x 2-bank pair tiles, shared by projections, the per-pack
    score/UM tile, and the output projection. The pack tile is written twice:
    scores land first, the exp evacuates them to SBUF, then the dpa/sden and
    att_mem/zden matmuls overwrite the same banks (WAR deps give the exact
    ordering the data flow needs anyway).
  - "small" 1 bank: per-pack sigma_k transpose scratch.
  - "state" 1 bank: persistent [mem(64)|z(1)] per head, all 4 packs packed
    on columns; the update matmuls accumulate in place (start=False) on top
    of a memset init, so no SBUF master copy and no vector adds.
"""

import os
import sys

for _p in ("/opt/trn_rl_repo",):
    if _p not in sys.path and os.path.isdir(_p):
        sys.path.insert(0, _p)

from contextlib import ExitStack

import ml_dtypes
import numpy as np

import concourse.bass as bass
import concourse.tile as tile
from concourse import bacc, mybir
from concourse.bass_utils import run_bass_kernel_spmd

AF = mybir.ActivationFunctionType
OP = mybir.AluOpType
F32 = mybir.dt.float32

B, S, D = 4, 8192, 1024
H, dk, dv, SEG = 16, 64, 64, 256
HL = 8  # heads per core
NCORES = 8

NSEG = int(os.environ.get("BASS_NSEG", S // SEG))
USE_BF16 = os.environ.get("BASS_CDT", "bf16") == "bf16"
CDT = mybir.dt.bfloat16 if USE_BF16 else F32
NPDT = ml_dtypes.bfloat16 if USE_BF16 else np.float32


def _emit(ctx, tc, nseg, xt_d, wq_d, wk_d, wv_d, wout_d, bsig_d, bsig1m_d, id128_d, out_d):
    nc = tc.nc

    consts = ctx.enter_context(tc.tile_pool(name="consts", bufs=1))
    state_p = ctx.enter_context(tc.tile_pool(name="state", bufs=1))
    xt_p = ctx.enter_context(tc.tile_pool(name="xtp", bufs=3))
    qk_p = ctx.enter_context(tc.tile_pool(name="qk", bufs=2))
    pt_p = ctx.enter_context(tc.tile_pool(name="ptp", bufs=4))
    at_p = ctx.enter_context(tc.tile_pool(name="atp", bufs=3))
    sm_p = ctx.enter_context(tc.tile_pool(name="smp", bufs=6))
    ob_p = ctx.enter_context(tc.tile_pool(name="obp", bufs=2))
    ps = ctx.enter_context(tc.tile_pool(name="ps", bufs=1, space="PSUM"))

    wq_sb = consts.tile([128, 8, 512], CDT, tag="wq")
    wk_sb = consts.tile([128, 8, 512], CDT, tag="wk")
    wv_sb = consts.tile([128, 8, 512], CDT, tag="wv")
    wout_sb = consts.tile([128, 8, 1024], CDT, tag="wout")
    nc.scalar.dma_start(wq_sb[:, 0:4, :], wq_d.ap()[:, 0:4, :])
    nc.gpsimd.dma_start(wq_sb[:, 4:8, :], wq_d.ap()[:, 4:8, :])
    nc.scalar.dma_start(wk_sb[:, 0:4, :], wk_d.ap()[:, 0:4, :])
    nc.gpsimd.dma_start(wk_sb[:, 4:8, :], wk_d.ap()[:, 4:8, :])
    nc.scalar.dma_start(wv_sb[:, 0:4, :], wv_d.ap()[:, 0:4, :])
    nc.gpsimd.dma_start(wv_sb[:, 4:8, :], wv_d.ap()[:, 4:8, :])
    nc.scalar.dma_start(wout_sb[:, 0:4, :], wout_d.ap()[:, 0:4, :])
    nc.gpsimd.dma_start(wout_sb[:, 4:8, :], wout_d.ap()[:, 4:8, :])
    bsig_sb = consts.tile([128, HL], F32, tag="bsig")
    bsig1m_sb = consts.tile([128, HL], F32, tag="bsig1m")
    nc.gpsimd.dma_start(bsig_sb[:], bsig_d.ap())
    nc.gpsimd.dma_start(bsig1m_sb[:], bsig1m_d.ap())
    id128 = consts.tile([128, 128], CDT, tag="id128")
    nc.scalar.dma_start(id128[:], id128_d.ap())
    ones128c = consts.tile([128, 128], CDT, tag="ones128c")
    nc.vector.memset(ones128c[:], 1.0)

    # persistent per-head memory state in PSUM: [mem(64) | z(1)], 4 packs on
    # columns of one bank; update matmuls accumulate in place forever.
    state_ps = ps.tile([128, 4, 65], F32, tag="state", bufs=1, name="state_ps")
    nc.vector.memset(state_ps[:, :, 0:64], 0.0)
    nc.vector.memset(state_ps[:, :, 64:65], 1.0 / dk)

    # stz: compute-dtype shadow as [z broadcast(64) | mem(64)] per pack so one
    # matmul yields the zden broadcast rows [0:64] and att_mem rows [64:128]
    stz_all = state_p.tile([128, 4, 128], CDT, tag="stz", name="stz_all")
    nc.vector.memset(stz_all[:, :, 0:64], 1.0 / dk)
    nc.vector.memset(stz_all[:, :, 64:128], 0.0)

    # vE: [ones(64) | v(64) | ones(1)] per (l-chunk, head); double-buffered
    # manually so the ones columns are written once, not per superseg
    vEs = [state_p.tile([128, 4, HL, 129], CDT, tag=f"vE_{i}", name=f"vE_{i}") for i in range(2)]
    for v in vEs:
        nc.vector.memset(v[:, :, :, 0:64], 1.0)
        nc.vector.memset(v[:, :, :, 128:129], 1.0)

    assert nseg % 2 == 0

    def emit_inputs(T):
        """Tiles + xt DMA for superseg T, with projections/elu as deferred
        closures so they can be woven into the previous superseg's schedule."""
        xt_sb = xt_p.tile([128, 8, 2, SEG], CDT, tag="xt", name="xt_sb")
        nc.sync.dma_start(xt_sb[:], xt_d.ap()[T])
        vE = vEs[T % 2]
        qt = qk_p.tile([128, 4, 512], CDT, tag="qt", name="qt")
        kt = qk_p.tile([128, 4, 512], CDT, tag="kt", name="kt")
        sq = qk_p.tile([128, 4, 512], CDT, tag="sq", name="sq")
        sk = qk_p.tile([128, 4, 512], CDT, tag="sk", name="sk")

        # ---- projections: qT,kT in [dk(2 heads), pack, l=512]; v natural ----
        projs = []
        for w_sb, dst in ((wq_sb, qt), (wk_sb, kt)):
            for pr in range(2):
                def mk_qk(w_sb=w_sb, dst=dst, pr=pr):
                    prjp = ps.tile([128, 2, 512], F32, tag="big", bufs=3, name="prjp")
                    for u in range(2):
                        pkk = 2 * pr + u
                        for kc in range(8):
                            nc.tensor.matmul(
                                prjp[:, u, :],
                                w_sb[:, kc, pkk * 128 : (pkk + 1) * 128],
                                xt_sb[:, kc, :, :],
                                start=(kc == 0),
                                stop=(kc == 7),
                            )
                    nc.scalar.copy(dst[:, 2 * pr : 2 * pr + 2, :], prjp[:])
                projs.append(mk_qk)
        for pr in range(2):
            def mk_v(pr=pr):
                prjp = ps.tile([128, 2, 512], F32, tag="big", bufs=3, name="prjv")
                for u in range(2):
                    c = 2 * pr + u
                    for kc in range(8):
                        nc.tensor.matmul(
                            prjp[:, u, :],
                            xt_sb[:, kc, c // 2, (c % 2) * 128 : (c % 2) * 128 + 128],
                            wv_sb[:, kc, :],
                            start=(kc == 0),
                            stop=(kc == 7),
                        )
                nc.scalar.copy(
                    vE[:, 2 * pr : 2 * pr + 2, :, 64:128],
                    prjp[:].rearrange("p u (h j) -> p u h j", h=HL),
                )
            projs.append(mk_v)

        # ---- elu(x)+1 = exp(min(x,0)) + max(x,0), one closure per segment
        def mk_elu(s):
            def emit():
                so = s * SEG
                for esrc, edst in ((qt, sq), (kt, sk)):
                    m0 = qk_p.tile([128, 4, SEG], CDT, tag="m0", name="m0")
                    ex = qk_p.tile([128, 4, SEG], CDT, tag="ex", name="ex")
                    nc.vector.tensor_scalar_min(m0[:], esrc[:, :, so : so + SEG], 0.0)
                    nc.scalar.activation(ex[:], m0[:], AF.Exp)
                    nc.vector.scalar_tensor_tensor(
                        edst[:, :, so : so + SEG], esrc[:, :, so : so + SEG], 0.0, ex[:],
                        op0=OP.max, op1=OP.add,
                    )
            return emit

        return dict(qt=qt, kt=kt, sq=sq, sk=sk, vE=vE, projs=projs,
                    elu=[mk_elu(0), mk_elu(1)])

    prev_out = None

    def emit_segment(cur, t, interleave, final=False):
        """One segment's attention. `interleave` is a list of closures (the
        next superseg's projection pieces) woven between stages so the PE
        always has dense work and stays at the high clock p-state."""
        nonlocal prev_out
        s = t % 2
        so = s * SEG
        qt, kt, sq, sk, vE = cur["qt"], cur["kt"], cur["sq"], cur["sk"], cur["vE"]
        # attS: att^T restacked for K=128 output projection.
        # rows [0:64] = att^T[:, l odd], rows [64:128] = att^T[:, l even]
        attS = at_p.tile([128, HL, 128], CDT, tag="attS", name="attS")

        def stage_a(pk):
            # UMp: 2-bank pair tile. Phase 1 (here): scores for both heads
            # (row-group paired). Phase 2 (stage_b): overwritten with
            # [dpa|sden] cols 0:256 and [zden|att_mem] cols 256:512.
            UMp = ps.tile([128, 2, 512], F32, tag="big", bufs=3, name="UMp")
            for mc in range(2):
                for u in range(2):
                    hs = u * 64
                    nc.tensor.matmul(
                        UMp[:, u, mc * SEG : (mc + 1) * SEG],
                        kt[hs : hs + 64, pk, so + mc * 128 : so + (mc + 1) * 128],
                        qt[hs : hs + 64, pk, so : so + SEG],
                        start=True,
                        stop=True,
                    )
            # sigma_k pack transpose: [128(dk,2h), 128(l)] -> [128(l), 128]
            trd = ps.tile([128, 2, 128], CDT, tag="small", bufs=1, name="trd")
            for mc in range(2):
                nc.tensor.transpose(
                    trd[:, mc, :],
                    sk[:, pk, so + mc * 128 : so + (mc + 1) * 128],
                    id128[:],
                )
            # sknP copy FIRST in the ACT queue: it frees the single-bank trd
            # slot that the very next stage_a's transposes spin on
            sknP = sm_p.tile([128, 2, 128], CDT, tag="sknP", bufs=3)
            nc.scalar.copy(sknP[:], trd[:])
            # P^T = exp(scores/8), per head so vE matmuls start sooner
            PT = pt_p.tile([128, 2, 2, SEG], CDT, tag="PT", name="PT")
            for u in range(2):
                nc.scalar.activation(
                    PT[:, u], UMp[:, u, :].rearrange("p (mc l) -> p mc l", mc=2),
                    AF.Exp, scale=0.125,
                )
            return (pk, UMp, PT, sknP)

        def stage_b(st):
            pk, UMp, PT, sknP = st
            # att_mem numerators + zden broadcast (K=64 row-group pair)
            for u in range(2):
                hs = u * 64
                nc.tensor.matmul(
                    UMp[:, u, 256:512],
                    stz_all[hs : hs + 64, pk, :],
                    sq[hs : hs + 64, pk, so : so + SEG],
                    start=True,
                    stop=True,
                )
            # zden ready: reciprocal + broadcast-to-numerator-rows early,
            # overlapping the dpa matmuls below
            rb2a = sm_p.tile([64, 2, 512], F32, tag="rb2a", bufs=2)
            rb2 = sm_p.tile([128, 2, 512], F32, tag="rb2", bufs=2)
            nc.vector.reciprocal_approx_fast(rb2a[:, :, 256:512], UMp[0:64, :, 256:512])
            nc.sync.dma_start(rb2[64:128, :, 256:512], rb2a[:, :, 256:512])
            # state accumulate first: it needs only sknP (ready early), so it
            # fills the PE while the exp producing PT drains on ACT
            for mc in range(2):
                for u in range(2):
                    hs = u * 64
                    nc.tensor.matmul(
                        state_ps[hs : hs + 64, pk, :],
                        sknP[:, mc, hs : hs + 64],
                        vE[:, 2 * s + mc, 2 * pk + u, 64:129],
                        start=False,
                        stop=False,
                        skip_group_check=True,
                    )
            # dpa rows [64:128] + sum_m P broadcast rows [0:64]
            for mc in range(2):
                for u in range(2):
                    nc.tensor.matmul(
                        UMp[:, u, 0:256],
                        vE[:, 2 * s + mc, 2 * pk + u, 0:128],
                        PT[:, u, mc, :],
                        start=(mc == 0),
                        stop=(mc == 1),
                    )
            # sden reciprocal + broadcast once the dpa matmuls finish
            nc.vector.reciprocal_approx_fast(rb2a[:, :, 0:256], UMp[0:64, :, 0:256])
            nc.sync.dma_start(rb2[64:128, :, 0:256], rb2a[:, :, 0:256])

            # combine: att = bsig * att_mem / zden + (1-bsig) * dpa / sden
            # bn first (needs only the early zden recip), t2 after rb-s
            bns, t2s = [], []
            for u in range(2):
                h = 2 * pk + u
                bn = sm_p.tile([128, SEG], F32, tag="bn", bufs=4, name="bn")
                bns.append(bn)
                nc.vector.scalar_tensor_tensor(
                    bn[64:128, :], UMp[64:128, u, 256:512], bsig_sb[64:128, h : h + 1],
                    rb2[64:128, u, 256:512], op0=OP.mult, op1=OP.mult,
                )
            for u in range(2):
                h = 2 * pk + u
                t2 = sm_p.tile([128, SEG], F32, tag="t2", bufs=4, name="t2")
                t2s.append(t2)
                nc.vector.scalar_tensor_tensor(
                    t2[64:128, :], UMp[64:128, u, 0:256], bsig1m_sb[64:128, h : h + 1],
                    rb2[64:128, u, 0:256], op0=OP.mult, op1=OP.mult,
                )
            for u in range(2):
                h = 2 * pk + u
                bne = bns[u][:].rearrange("p (a two) -> p a two", two=2)
                t2e = t2s[u][:].rearrange("p (a two) -> p a two", two=2)
                nc.gpsimd.tensor_add(attS[64:128, h, :], bne[64:128, :, 0], t2e[64:128, :, 0])
                bo = sm_p.tile([128, 128], CDT, tag="bo", bufs=4, name="bo")
                nc.gpsimd.tensor_add(bo[64:128, :], bne[64:128, :, 1], t2e[64:128, :, 1])
                nc.sync.dma_start(attS[0:64, h, :], bo[64:128, :])

        def weave(i):
            if i < len(interleave):
                interleave[i]()

        # software-pipelined pack loop (depth 2); the previous segment's
        # output projection and the next superseg's projection pieces are
        # woven in so PE work stays dense through the combine tails
        st0 = stage_a(0)
        st1 = stage_a(1)
        if prev_out is not None:
            prev_out()
            prev_out = None
        st2 = stage_a(2)
        stage_b(st0)
        weave(0)
        st3 = stage_a(3)
        weave(1)
        stage_b(st1)
        weave(2)
        if final:
            # last segment: no later work to hide the outproj behind, so emit
            # it in head-halves as the combines complete (M=64 column groups)
            fin_o = ob_p.tile([128, 2, 512], F32, tag="o", name="fin_o")
            fin_aT = attS[:].rearrange("p h (lh lmo) -> p h lh lmo", lmo=8)
            fin_pot = ps.tile([128, 2, 512], F32, tag="big", bufs=3, name="fpot")
            for nh in range(2):
                for lm0 in range(8):
                    nc.tensor.matmul(
                        fin_pot[0:64, nh, :], fin_aT[:, 0:4, :, lm0],
                        wout_sb[:, lm0, nh * 512 : (nh + 1) * 512],
                        start=(lm0 == 0), stop=(lm0 == 7),
                    )
        stage_b(st2)
        weave(3)
        stage_b(st3)
        for i in range(4, len(interleave)):
            interleave[i]()
        if final:
            for nh in range(2):
                for lm0 in range(8):
                    nc.tensor.matmul(
                        fin_pot[64:128, nh, :], fin_aT[:, 4:8, :, lm0],
                        wout_sb[:, lm0, nh * 512 : (nh + 1) * 512],
                        start=(lm0 == 0), stop=(lm0 == 7),
                    )
            nc.scalar.copy(fin_o[:], fin_pot[:])
            nc.sync.dma_start(
                out_d.ap()[t * 128 : (t + 1) * 128, :],
                fin_o[:].rearrange("p u c -> p (u c)"),
            )

        # refresh the compute-dtype state shadow for all packs at once
        # (ACT scale port only reads SBUF, so stage z there first)
        zsb = sm_p.tile([128, 4], F32, tag="zsb", bufs=2)
        nc.scalar.copy(zsb[:], state_ps[:, :, 64])
        for pk in range(4):
            nc.scalar.mul(stz_all[:, pk, 0:64], ones128c[:, 0:64], zsb[:, pk : pk + 1])
        nc.scalar.copy(stz_all[:, :, 64:128], state_ps[:, :, 0:64])

        def make_outproj(attS, t):
            def emit():
                # out_rows = att_view @ Wout (raw-view structure: 8 rank-128 updates)
                o_sb = ob_p.tile([128, 2, 512], F32, tag="o")
                aT = attS[:].rearrange("p h (lh lmo) -> p h lh lmo", lmo=8)
                pot = ps.tile([128, 2, 512], F32, tag="big", bufs=3, name="pot")
                for nh in range(2):
                    for lm0 in range(8):
                        rhs = wout_sb[:, lm0, nh * 512 : (nh + 1) * 512]
                        nc.tensor.matmul(
                            pot[:, nh, :], aT[:, :, :, lm0], rhs, start=(lm0 == 0), stop=(lm0 == 7)
                        )
                nc.scalar.copy(o_sb[:], pot[:])
                nc.sync.dma_start(
                    out_d.ap()[t * 128 : (t + 1) * 128, :],
                    o_sb[:].rearrange("p u c -> p (u c)"),
                )

            return emit

        prev_out = None if final else make_outproj(attS, t)

    # BASS_WEAVE: 0 = projections at superseg top; N>0 = weave the first N
    # next-superseg projection pieces into seg1's attention schedule
    weave_n = int(os.environ.get("BASS_WEAVE", "0"))
    cur = emit_inputs(0)
    for p in cur["projs"]:
        p()
    cur["elu"][0]()
    for T in range(nseg // 2):
        emit_segment(cur, 2 * T, [])
        cur["elu"][1]()
        if T + 1 < nseg // 2:
            if weave_n > 0:
                nxt = emit_inputs(T + 1)
                pieces = nxt["projs"] + [nxt["elu"][0]]
                emit_segment(cur, 2 * T + 1, pieces[:weave_n])
                for p in pieces[weave_n:]:
                    p()
            else:
                emit_segment(cur, 2 * T + 1, [])
                nxt = emit_inputs(T + 1)
                for p in nxt["projs"]:
                    p()
                nxt["elu"][0]()
            cur = nxt
        else:
            emit_segment(cur, 2 * T + 1, [], final=True)
    assert prev_out is None


def build_program(nseg=NSEG):
    nc = bacc.Bacc("TRN2", target_bir_lowering=False, debug=False, num_devices=NCORES)
    xt_d = nc.dram_tensor("xt", [nseg // 2, 128, 8, 2, SEG], CDT, kind="ExternalInput")
    wq_d = nc.dram_tensor("wq", [128, 8, 512], CDT, kind="ExternalInput")
    wk_d = nc.dram_tensor("wk", [128, 8, 512], CDT, kind="ExternalInput")
    wv_d = nc.dram_tensor("wv", [128, 8, 512], CDT, kind="ExternalInput")
    wout_d = nc.dram_tensor("wout", [128, 8, 1024], CDT, kind="ExternalInput")
    bsig_d = nc.dram_tensor("bsig", [128, HL], F32, kind="ExternalInput")
    bsig1m_d = nc.dram_tensor("bsig1m", [128, HL], F32, kind="ExternalInput")
    id128_d = nc.dram_tensor("id128", [128, 128], CDT, kind="ExternalInput")
    out_d = nc.dram_tensor("out", [nseg * 128, 1024], F32, kind="ExternalOutput")
    with tile.TileContext(nc) as tc:
        with ExitStack() as ctx:
            _emit(ctx, tc, nseg, xt_d, wq_d, wk_d, wv_d, wout_d, bsig_d, bsig1m_d, id128_d, out_d)
    nc.compile()
    return nc


def shard_inputs(x, Wq, Wk, Wv, Wout, betas, nseg=NSEG):
    x = np.asarray(x, np.float32)
    Wq = np.asarray(Wq, np.float32)
    Wk = np.asarray(Wk, np.float32)
    Wv = np.asarray(Wv, np.float32)
    Wout = np.asarray(Wout, np.float32)
    betas = np.asarray(betas, np.float32)
    sig = 1.0 / (1.0 + np.exp(-betas[0, :, 0, :]))  # [H, dv]

    w8 = Wout.reshape(8, 128, 1024)
    wout_t = np.ascontiguousarray(
        np.concatenate([w8[:, 64:128], w8[:, 0:64]], axis=1).transpose(1, 0, 2)
    ).astype(NPDT)
    id128 = np.eye(128, dtype=np.float32).astype(NPDT)
    in_maps = []
    for c in range(NCORES):
        b, hg = c // 2, c % 2
        hb = hg * HL
        xt = x[b].T.reshape(8, 128, S // SEG // 2, 2, SEG).transpose(2, 1, 0, 3, 4)[: nseg // 2]
        m = {
            "xt": np.ascontiguousarray(xt).astype(NPDT),
            "wq": np.ascontiguousarray(Wq[:, hb * 64 : (hb + HL) * 64].reshape(8, 128, 512).transpose(1, 0, 2)).astype(NPDT),
            "wk": np.ascontiguousarray(Wk[:, hb * 64 : (hb + HL) * 64].reshape(8, 128, 512).transpose(1, 0, 2)).astype(NPDT),
            "wv": np.ascontiguousarray(Wv[:, hb * 64 : (hb + HL) * 64].reshape(8, 128, 512).transpose(1, 0, 2)).astype(NPDT),
            "wout": wout_t,
            "bsig": np.ascontiguousarray(np.tile(sig[hb : hb + HL].T, (2, 1))),
            "bsig1m": np.ascontiguousarray(np.tile((1.0 - sig)[hb : hb + HL].T, (2, 1))),
            "id128": id128,
        }
        in_maps.append(m)
    return in_maps


def assemble_output(results, nseg=NSEG):
    out = np.empty((B, nseg * SEG, D), np.float32)
    o5 = out.reshape(B, nseg, 2, 128, D)
    for c in range(NCORES):
        b, hg = c // 2, c % 2
        o5[b, :, hg] = results[c]["out"].reshape(nseg, 128, D)
    return out


_COMPILED = {}


def _get_program(nseg=NSEG):
    if nseg not in _COMPILED:
        _COMPILED[nseg] = build_program(nseg)
    return _COMPILED[nseg]


def run(x, Wq, Wk, Wv, Wout, betas, nseg=NSEG, trace=False, tmpdir=None):
    nc = _get_program(nseg)
    in_maps = shard_inputs(x, Wq, Wk, Wv, Wout, betas, nseg)
    res = run_bass_kernel_spmd(
        nc, in_maps, list(range(NCORES)), trace=trace, tmpdir=tmpdir
    )
    return assemble_output(res.results, nseg), res.exec_time_ns


def kernel(x, Wq, Wk, Wv, Wout, betas):
    out, _ = run(x, Wq, Wk, Wv, Wout, betas, nseg=NSEG, trace=False)
    return out


# revision 47
# speedup vs baseline: 1.0021x; 1.0021x over previous
"""Trainium2 Bass kernel for CompressiveMemory (Infini-attention style).

Sharding: 8 cores = 4 batch x 2 head-groups (8 heads each). The reference's
`att.reshape(B, SEG, H*dv)` is a raw view of a (B,H,SEG,dv) tensor, so each
block of 16 output rows depends on exactly one head: head-sharding needs no
cross-core reduction, only row scattering (done on host).

Per-core kernel. Layouts put matmul contractions on the partition dim; heads
are processed in pairs (even head on partitions 0:64, odd on 64:128) so the
K=64 matmuls (scores, att_mem) auto-place into PE row groups 0/64 and
co-execute, and the M=64 state updates co-execute via column groups.

PSUM (8 banks):
  - "big" ring, 3 x 2-bank pair tiles, shared by projections, the per-pack
    score/UM tile, and the output projection. The pack tile is written twice:
    scores land first, the exp evacuates them to SBUF, then the dpa/sden and
    att_mem/zden matmuls overwrite the same banks (WAR deps give the exact
    ordering the data flow needs anyway).
  - "small" 1 bank: per-pack sigma_k transpose scratch.
  - "state" 1 bank: persistent [mem(64)|z(1)] per head, all 4 packs packed
    on columns; the update matmuls accumulate in place (start=False) on top
    of a memset init, so no SBUF master copy and no vector adds.
"""

import os
import sys

for _p in ("/opt/trn_rl_repo",):
    if _p not in sys.path and os.path.isdir(_p):
        sys.path.insert(0, _p)

from contextlib import ExitStack

import ml_dtypes
import numpy as np

import concourse.bass as bass
import concourse.tile as tile
from concourse import bacc, mybir
from concourse.bass_utils import run_bass_kernel_spmd

AF = mybir.ActivationFunctionType
OP = mybir.AluOpType
F32 = mybir.dt.float32

B, S, D = 4, 8192, 1024
H, dk, dv, SEG = 16, 64, 64, 256
HL = 8  # heads per core
NCORES = 8

NSEG = int(os.environ.get("BASS_NSEG", S // SEG))
USE_BF16 = os.environ.get("BASS_CDT", "bf16") == "bf16"
CDT = mybir.dt.bfloat16 if USE_BF16 else F32
NPDT = ml_dtypes.bfloat16 if USE_BF16 else np.float32


def _emit(ctx, tc, nseg, xt_d, wq_d, wk_d, wv_d, wout_d, bsig_d, bsig1m_d, id128_d, out_d):
    nc = tc.nc

    consts = ctx.enter_context(tc.tile_pool(name="consts", bufs=1))
    state_p = ctx.enter_context(tc.tile_pool(name="state", bufs=1))
    xt_p = ctx.enter_context(tc.tile_pool(name="xtp", bufs=3))
    qk_p = ctx.enter_context(tc.tile_pool(name="qk", bufs=2))
    pt_p = ctx.enter_context(tc.tile_pool(name="ptp", bufs=4))
    at_p = ctx.enter_context(tc.tile_pool(name="atp", bufs=3))
    sm_p = ctx.enter_context(tc.tile_pool(name="smp", bufs=6))
    ob_p = ctx.enter_context(tc.tile_pool(name="obp", bufs=2))
    ps = ctx.enter_context(tc.tile_pool(name="ps", bufs=1, space="PSUM"))

    wq_sb = consts.tile([128, 8, 512], CDT, tag="wq")
    wk_sb = consts.tile([128, 8, 512], CDT, tag="wk")
    wv_sb = consts.tile([128, 8, 512], CDT, tag="wv")
    wout_sb = consts.tile([128, 8, 1024], CDT, tag="wout")
    nc.scalar.dma_start(wq_sb[:, 0:4, :], wq_d.ap()[:, 0:4, :])
    nc.gpsimd.dma_start(wq_sb[:, 4:8, :], wq_d.ap()[:, 4:8, :])
    nc.scalar.dma_start(wk_sb[:, 0:4, :], wk_d.ap()[:, 0:4, :])
    nc.gpsimd.dma_start(wk_sb[:, 4:8, :], wk_d.ap()[:, 4:8, :])
    nc.scalar.dma_start(wv_sb[:, 0:4, :], wv_d.ap()[:, 0:4, :])
    nc.gpsimd.dma_start(wv_sb[:, 4:8, :], wv_d.ap()[:, 4:8, :])
    nc.scalar.dma_start(wout_sb[:, 0:4, :], wout_d.ap()[:, 0:4, :])
    nc.gpsimd.dma_start(wout_sb[:, 4:8, :], wout_d.ap()[:, 4:8, :])
    bsig_sb = consts.tile([128, HL], F32, tag="bsig")
    bsig1m_sb = consts.tile([128, HL], F32, tag="bsig1m")
    nc.gpsimd.dma_start(bsig_sb[:], bsig_d.ap())
    nc.gpsimd.dma_start(bsig1m_sb[:], bsig1m_d.ap())
    id128 = consts.tile([128, 128], CDT, tag="id128")
    nc.scalar.dma_start(id128[:], id128_d.ap())
    ones128c = consts.tile([128, 128], CDT, tag="ones128c")
    nc.vector.memset(ones128c[:], 1.0)

    # persistent per-head memory state in PSUM: [mem(64) | z(1)], 4 packs on
    # columns of one bank; update matmuls accumulate in place forever.
    state_ps = ps.tile([128, 4, 65], F32, tag="state", bufs=1, name="state_ps")
    nc.vector.memset(state_ps[:, :, 0:64], 0.0)
    nc.vector.memset(state_ps[:, :, 64:65], 1.0 / dk)

    # stz: compute-dtype shadow as [z broadcast(64) | mem(64)] per pack so one
    # matmul yields the zden broadcast rows [0:64] and att_mem rows [64:128]
    stz_all = state_p.tile([128, 4, 128], CDT, tag="stz", name="stz_all")
    nc.vector.memset(stz_all[:, :, 0:64], 1.0 / dk)
    nc.vector.memset(stz_all[:, :, 64:128], 0.0)

    # vE: [ones(64) | v(64) | ones(1)] per (l-chunk, head); double-buffered
    # manually so the ones columns are written once, not per superseg
    vEs = [state_p.tile([128, 4, HL, 129], CDT, tag=f"vE_{i}", name=f"vE_{i}") for i in range(2)]
    for v in vEs:
        nc.vector.memset(v[:, :, :, 0:64], 1.0)
        nc.vector.memset(v[:, :, :, 128:129], 1.0)

    assert nseg % 2 == 0

    def emit_inputs(T):
        """Tiles + xt DMA for superseg T, with projections/elu as deferred
        closures so they can be woven into the previous superseg's schedule."""
        xt_sb = xt_p.tile([128, 8, 2, SEG], CDT, tag="xt", name="xt_sb")
        nc.sync.dma_start(xt_sb[:], xt_d.ap()[T])
        vE = vEs[T % 2]
        qt = qk_p.tile([128, 4, 512], CDT, tag="qt", name="qt")
        kt = qk_p.tile([128, 4, 512], CDT, tag="kt", name="kt")
        sq = qk_p.tile([128, 4, 512], CDT, tag="sq", name="sq")
        sk = qk_p.tile([128, 4, 512], CDT, tag="sk", name="sk")

        # ---- projections: qT,kT in [dk(2 heads), pack, l=512]; v natural ----
        projs = []
        for w_sb, dst in ((wq_sb, qt), (wk_sb, kt)):
            for pr in range(2):
                def mk_qk(w_sb=w_sb, dst=dst, pr=pr):
                    prjp = ps.tile([128, 2, 512], F32, tag="big", bufs=3, name="prjp")
                    for u in range(2):
                        pkk = 2 * pr + u
                        for kc in range(8):
                            nc.tensor.matmul(
                                prjp[:, u, :],
                                w_sb[:, kc, pkk * 128 : (pkk + 1) * 128],
                                xt_sb[:, kc, :, :],
                                start=(kc == 0),
                                stop=(kc == 7),
                            )
                    nc.scalar.copy(dst[:, 2 * pr : 2 * pr + 2, :], prjp[:])
                projs.append(mk_qk)
        for pr in range(2):
            def mk_v(pr=pr):
                prjp = ps.tile([128, 2, 512], F32, tag="big", bufs=3, name="prjv")
                for u in range(2):
                    c = 2 * pr + u
                    for kc in range(8):
                        nc.tensor.matmul(
                            prjp[:, u, :],
                            xt_sb[:, kc, c // 2, (c % 2) * 128 : (c % 2) * 128 + 128],
                            wv_sb[:, kc, :],
                            start=(kc == 0),
                            stop=(kc == 7),
                        )
                nc.scalar.copy(
                    vE[:, 2 * pr : 2 * pr + 2, :, 64:128],
                    prjp[:].rearrange("p u (h j) -> p u h j", h=HL),
                )
            projs.append(mk_v)

        # ---- elu(x)+1 = exp(min(x,0)) + max(x,0), one closure per segment
        def mk_elu(s):
            def emit():
                so = s * SEG
                for esrc, edst in ((qt, sq), (kt, sk)):
                    m0 = qk_p.tile([128, 4, SEG], CDT, tag="m0", name="m0")
                    ex = qk_p.tile([128, 4, SEG], CDT, tag="ex", name="ex")
                    nc.vector.tensor_scalar_min(m0[:], esrc[:, :, so : so + SEG], 0.0)
                    nc.scalar.activation(ex[:], m0[:], AF.Exp)
                    nc.vector.scalar_tensor_tensor(
                        edst[:, :, so : so + SEG], esrc[:, :, so : so + SEG], 0.0, ex[:],
                        op0=OP.max, op1=OP.add,
                    )
            return emit

        return dict(qt=qt, kt=kt, sq=sq, sk=sk, vE=vE, projs=projs,
                    elu=[mk_elu(0), mk_elu(1)])

    prev_out = None

    def emit_segment(cur, t, interleave, final=False):
        """One segment's attention. `interleave` is a list of closures (the
        next superseg's projection pieces) woven between stages so the PE
        always has dense work and stays at the high clock p-state."""
        nonlocal prev_out
        s = t % 2
        so = s * SEG
        qt, kt, sq, sk, vE = cur["qt"], cur["kt"], cur["sq"], cur["sk"], cur["vE"]
        # attS: att^T restacked for K=128 output projection.
        # rows [0:64] = att^T[:, l odd], rows [64:128] = att^T[:, l even]
        attS = at_p.tile([128, HL, 128], CDT, tag="attS", name="attS")

        def stage_a(pk):
            # UMp: 2-bank pair tile. Phase 1 (here): scores for both heads
            # (row-group paired). Phase 2 (stage_b): overwritten with
            # [dpa|sden] cols 0:256 and [zden|att_mem] cols 256:512.
            UMp = ps.tile([128, 2, 512], F32, tag="big", bufs=3, name="UMp")
            for mc in range(2):
                for u in range(2):
                    hs = u * 64
                    nc.tensor.matmul(
                        UMp[:, u, mc * SEG : (mc + 1) * SEG],
                        kt[hs : hs + 64, pk, so + mc * 128 : so + (mc + 1) * 128],
                        qt[hs : hs + 64, pk, so : so + SEG],
                        start=True,
                        stop=True,
                    )
            # sigma_k pack transpose: [128(dk,2h), 128(l)] -> [128(l), 128]
            trd = ps.tile([128, 2, 128], CDT, tag="small", bufs=1, name="trd")
            for mc in range(2):
                nc.tensor.transpose(
                    trd[:, mc, :],
                    sk[:, pk, so + mc * 128 : so + (mc + 1) * 128],
                    id128[:],
                )
            # sknP copy FIRST in the ACT queue: it frees the single-bank trd
            # slot that the very next stage_a's transposes spin on
            sknP = sm_p.tile([128, 2, 128], CDT, tag="sknP", bufs=3)
            nc.scalar.copy(sknP[:], trd[:])
            # P^T = exp(scores/8), per head so vE matmuls start sooner
            PT = pt_p.tile([128, 2, 2, SEG], CDT, tag="PT", name="PT")
            for u in range(2):
                nc.scalar.activation(
                    PT[:, u], UMp[:, u, :].rearrange("p (mc l) -> p mc l", mc=2),
                    AF.Exp, scale=0.125,
                )
            return (pk, UMp, PT, sknP)

        def stage_b(st):
            pk, UMp, PT, sknP = st
            # att_mem numerators + zden broadcast (K=64 row-group pair)
            for u in range(2):
                hs = u * 64
                nc.tensor.matmul(
                    UMp[:, u, 256:512],
                    stz_all[hs : hs + 64, pk, :],
                    sq[hs : hs + 64, pk, so : so + SEG],
                    start=True,
                    stop=True,
                )
            # zden ready: reciprocal + broadcast-to-numerator-rows early,
            # overlapping the dpa matmuls below
            rb2a = sm_p.tile([64, 2, 512], F32, tag="rb2a", bufs=2)
            rb2 = sm_p.tile([128, 2, 512], F32, tag="rb2", bufs=2)
            nc.vector.reciprocal_approx_fast(rb2a[:, :, 256:512], UMp[0:64, :, 256:512])
            nc.sync.dma_start(rb2[64:128, :, 256:512], rb2a[:, :, 256:512])
            # dpa rows [64:128] + sum_m P broadcast rows [0:64]
            for mc in range(2):
                for u in range(2):
                    nc.tensor.matmul(
                        UMp[:, u, 0:256],
                        vE[:, 2 * s + mc, 2 * pk + u, 0:128],
                        PT[:, u, mc, :],
                        start=(mc == 0),
                        stop=(mc == 1),
                    )
            # state accumulate: mem += sigma_k^T v ; z += sum_l sigma_k
            # (M=64 column-group pair, accumulating in place forever)
            for mc in range(2):
                for u in range(2):
                    hs = u * 64
                    nc.tensor.matmul(
                        state_ps[hs : hs + 64, pk, :],
                        sknP[:, mc, hs : hs + 64],
                        vE[:, 2 * s + mc, 2 * pk + u, 64:129],
                        start=False,
                        stop=False,
                        skip_group_check=True,
                    )
            # sden reciprocal + broadcast once the dpa matmuls finish
            nc.vector.reciprocal_approx_fast(rb2a[:, :, 0:256], UMp[0:64, :, 0:256])
            nc.sync.dma_start(rb2[64:128, :, 0:256], rb2a[:, :, 0:256])

            # combine: att = bsig * att_mem / zden + (1-bsig) * dpa / sden
            # bn first (needs only the early zden recip), t2 after rb-s
            bns, t2s = [], []
            for u in range(2):
                h = 2 * pk + u
                bn = sm_p.tile([128, SEG], F32, tag="bn", bufs=4, name="bn")
                bns.append(bn)
                nc.vector.scalar_tensor_tensor(
                    bn[64:128, :], UMp[64:128, u, 256:512], bsig_sb[64:128, h : h + 1],
                    rb2[64:128, u, 256:512], op0=OP.mult, op1=OP.mult,
                )
            for u in range(2):
                h = 2 * pk + u
                t2 = sm_p.tile([128, SEG], F32, tag="t2", bufs=4, name="t2")
                t2s.append(t2)
                nc.vector.scalar_tensor_tensor(
                    t2[64:128, :], UMp[64:128, u, 0:256], bsig1m_sb[64:128, h : h + 1],
                    rb2[64:128, u, 0:256], op0=OP.mult, op1=OP.mult,
                )
            for u in range(2):
                h = 2 * pk + u
                bne = bns[u][:].rearrange("p (a two) -> p a two", two=2)
                t2e = t2s[u][:].rearrange("p (a two) -> p a two", two=2)
                nc.gpsimd.tensor_add(attS[64:128, h, :], bne[64:128, :, 0], t2e[64:128, :, 0])
                bo = sm_p.tile([128, 128], CDT, tag="bo", bufs=4, name="bo")
                nc.gpsimd.tensor_add(bo[64:128, :], bne[64:128, :, 1], t2e[64:128, :, 1])
                nc.sync.dma_start(attS[0:64, h, :], bo[64:128, :])

        def weave(i):
            if i < len(interleave):
                interleave[i]()

        # software-pipelined pack loop (depth 2); the previous segment's
        # output projection and the next superseg's projection pieces are
        # woven in so PE work stays dense through the combine tails
        st0 = stage_a(0)
        st1 = stage_a(1)
        if prev_out is not None:
            prev_out()
            prev_out = None
        st2 = stage_a(2)
        stage_b(st0)
        weave(0)
        st3 = stage_a(3)
        weave(1)
        stage_b(st1)
        weave(2)
        if final:
            # last segment: no later work to hide the outproj behind, so emit
            # it in head-halves as the combines complete (M=64 column groups)
            fin_o = ob_p.tile([128, 2, 512], F32, tag="o", name="fin_o")
            fin_aT = attS[:].rearrange("p h (lh lmo) -> p h lh lmo", lmo=8)
            fin_pot = ps.tile([128, 2, 512], F32, tag="big", bufs=3, name="fpot")
            for nh in range(2):
                for lm0 in range(8):
                    nc.tensor.matmul(
                        fin_pot[0:64, nh, :], fin_aT[:, 0:4, :, lm0],
                        wout_sb[:, lm0, nh * 512 : (nh + 1) * 512],
                        start=(lm0 == 0), stop=(lm0 == 7),
                    )
        stage_b(st2)
        weave(3)
        stage_b(st3)
        for i in range(4, len(interleave)):
            interleave[i]()
        if final:
            for nh in range(2):
                for lm0 in range(8):
                    nc.tensor.matmul(
                        fin_pot[64:128, nh, :], fin_aT[:, 4:8, :, lm0],
                        wout_sb[:, lm0, nh * 512 : (nh + 1) * 512],
                        start=(lm0 == 0), stop=(lm0 == 7),
                    )
            nc.scalar.copy(fin_o[:], fin_pot[:])
            nc.sync.dma_start(
                out_d.ap()[t * 128 : (t + 1) * 128, :],
                fin_o[:].rearrange("p u c -> p (u c)"),
            )

        # refresh the compute-dtype state shadow for all packs at once
        # (ACT scale port only reads SBUF, so stage z there first)
        zsb = sm_p.tile([128, 4], F32, tag="zsb", bufs=2)
        nc.scalar.copy(zsb[:], state_ps[:, :, 64])
        for pk in range(4):
            nc.scalar.mul(stz_all[:, pk, 0:64], ones128c[:, 0:64], zsb[:, pk : pk + 1])
        nc.scalar.copy(stz_all[:, :, 64:128], state_ps[:, :, 0:64])

        def make_outproj(attS, t):
            def emit():
                # out_rows = att_view @ Wout (raw-view structure: 8 rank-128 updates)
                o_sb = ob_p.tile([128, 2, 512], F32, tag="o")
                aT = attS[:].rearrange("p h (lh lmo) -> p h lh lmo", lmo=8)
                pot = ps.tile([128, 2, 512], F32, tag="big", bufs=3, name="pot")
                for nh in range(2):
                    for lm0 in range(8):
                        rhs = wout_sb[:, lm0, nh * 512 : (nh + 1) * 512]
                        nc.tensor.matmul(
                            pot[:, nh, :], aT[:, :, :, lm0], rhs, start=(lm0 == 0), stop=(lm0 == 7)
                        )
                nc.scalar.copy(o_sb[:], pot[:])
                nc.sync.dma_start(
                    out_d.ap()[t * 128 : (t + 1) * 128, :],
                    o_sb[:].rearrange("p u c -> p (u c)"),
                )

            return emit

        prev_out = None if final else make_outproj(attS, t)

    # BASS_WEAVE: 0 = projections at superseg top; N>0 = weave the first N
    # next-superseg projection pieces into seg1's attention schedule
    weave_n = int(os.environ.get("BASS_WEAVE", "0"))
    cur = emit_inputs(0)
    for p in cur["projs"]:
        p()
    cur["elu"][0]()
    for T in range(nseg // 2):
        emit_segment(cur, 2 * T, [])
        cur["elu"][1]()
        if T + 1 < nseg // 2:
            if weave_n > 0:
                nxt = emit_inputs(T + 1)
                pieces = nxt["projs"] + [nxt["elu"][0]]
                emit_segment(cur, 2 * T + 1, pieces[:weave_n])
                for p in pieces[weave_n:]:
                    p()
            else:
                emit_segment(cur, 2 * T + 1, [])
                nxt = emit_inputs(T + 1)
                for p in nxt["projs"]:
                    p()
                nxt["elu"][0]()
            cur = nxt
        else:
            emit_segment(cur, 2 * T + 1, [], final=True)
    assert prev_out is None


def build_program(nseg=NSEG):
    nc = bacc.Bacc("TRN2", target_bir_lowering=False, debug=False, num_devices=NCORES)
    xt_d = nc.dram_tensor("xt", [nseg // 2, 128, 8, 2, SEG], CDT, kind="ExternalInput")
    wq_d = nc.dram_tensor("wq", [128, 8, 512], CDT, kind="ExternalInput")
    wk_d = nc.dram_tensor("wk", [128, 8, 512], CDT, kind="ExternalInput")
    wv_d = nc.dram_tensor("wv", [128, 8, 512], CDT, kind="ExternalInput")
    wout_d = nc.dram_tensor("wout", [128, 8, 1024], CDT, kind="ExternalInput")
    bsig_d = nc.dram_tensor("bsig", [128, HL], F32, kind="ExternalInput")
    bsig1m_d = nc.dram_tensor("bsig1m", [128, HL], F32, kind="ExternalInput")
    id128_d = nc.dram_tensor("id128", [128, 128], CDT, kind="ExternalInput")
    out_d = nc.dram_tensor("out", [nseg * 128, 1024], F32, kind="ExternalOutput")
    with tile.TileContext(nc) as tc:
        with ExitStack() as ctx:
            _emit(ctx, tc, nseg, xt_d, wq_d, wk_d, wv_d, wout_d, bsig_d, bsig1m_d, id128_d, out_d)
    nc.compile()
    return nc


def shard_inputs(x, Wq, Wk, Wv, Wout, betas, nseg=NSEG):
    x = np.asarray(x, np.float32)
    Wq = np.asarray(Wq, np.float32)
    Wk = np.asarray(Wk, np.float32)
    Wv = np.asarray(Wv, np.float32)
    Wout = np.asarray(Wout, np.float32)
    betas = np.asarray(betas, np.float32)
    sig = 1.0 / (1.0 + np.exp(-betas[0, :, 0, :]))  # [H, dv]

    w8 = Wout.reshape(8, 128, 1024)
    wout_t = np.ascontiguousarray(
        np.concatenate([w8[:, 64:128], w8[:, 0:64]], axis=1).transpose(1, 0, 2)
    ).astype(NPDT)
    id128 = np.eye(128, dtype=np.float32).astype(NPDT)
    in_maps = []
    for c in range(NCORES):
        b, hg = c // 2, c % 2
        hb = hg * HL
        xt = x[b].T.reshape(8, 128, S // SEG // 2, 2, SEG).transpose(2, 1, 0, 3, 4)[: nseg // 2]
        m = {
            "xt": np.ascontiguousarray(xt).astype(NPDT),
            "wq": np.ascontiguousarray(Wq[:, hb * 64 : (hb + HL) * 64].reshape(8, 128, 512).transpose(1, 0, 2)).astype(NPDT),
            "wk": np.ascontiguousarray(Wk[:, hb * 64 : (hb + HL) * 64].reshape(8, 128, 512).transpose(1, 0, 2)).astype(NPDT),
            "wv": np.ascontiguousarray(Wv[:, hb * 64 : (hb + HL) * 64].reshape(8, 128, 512).transpose(1, 0, 2)).astype(NPDT),
            "wout": wout_t,
            "bsig": np.ascontiguousarray(np.tile(sig[hb : hb + HL].T, (2, 1))),
            "bsig1m": np.ascontiguousarray(np.tile((1.0 - sig)[hb : hb + HL].T, (2, 1))),
            "id128": id128,
        }
        in_maps.append(m)
    return in_maps


def assemble_output(results, nseg=NSEG):
    out = np.empty((B, nseg * SEG, D), np.float32)
    o5 = out.reshape(B, nseg, 2, 128, D)
    for c in range(NCORES):
        b, hg = c // 2, c % 2
        o5[b, :, hg] = results[c]["out"].reshape(nseg, 128, D)
    return out


_COMPILED = {}


def _get_program(nseg=NSEG):
    if nseg not in _COMPILED:
        _COMPILED[nseg] = build_program(nseg)
    return _COMPILED[nseg]


def run(x, Wq, Wk, Wv, Wout, betas, nseg=NSEG, trace=False, tmpdir=None):
    nc = _get_program(nseg)
    in_maps = shard_inputs(x, Wq, Wk, Wv, Wout, betas, nseg)
    res = run_bass_kernel_spmd(
        nc, in_maps, list(range(NCORES)), trace=trace, tmpdir=tmpdir
    )
    return assemble_output(res.results, nseg), res.exec_time_ns


def kernel(x, Wq, Wk, Wv, Wout, betas):
    out, _ = run(x, Wq, Wk, Wv, Wout, betas, nseg=NSEG, trace=False)
    return out
